# revision 8
# baseline (speedup 1.0000x reference)
"""DualGNNModel Trainium2 kernel (8 NeuronCores, Bass/Tile).

Self-contained: accepts FULL inputs (as reference.setup_inputs()), returns the
FULL [256, 1] float32 output.

Sharding: cores 0-3 run the solute GCN encoder, cores 4-7 the solvent encoder
(graph-level model parallelism over the two independent encoders). Within each
4-core group, edges are partitioned by destination node into 4 contiguous
12800-node ranges; GCN/MLP weights are replicated. Per layer each core:
  dense   b = h @ W                  (replicated over the group, fp16)
  gather  g = b[src] rows            (dma_gather, two int16 half-table passes,
                                      fp16 rows, round-robin over 4 SWDGE
                                      queues for HW DMA parallelism)
  scale   g *= nsrc[src]*ndst[dst]   (symmetric GCN norm folded per edge)
  scatter psum[feat, dstwin] += g_chunk^T @ onehot(dst)    (fp16 PE matmuls)
  bias    psum += outer(bias, ones);  hT_own = relu(psum)
h shards are AllGather'd between layers; after layer 3 each core pools its own
shard via a membership-one-hot matmul, partial pools are AllReduce'd within the
group, the two encoders' pooled embeddings are exchanged pairwise, and the
(tiny) MLP head runs replicated on every core.

The edge datapath is fp16 end to end (gather table, edge norms, one-hot
destination masks, PE scatter matmuls): fp16 matmuls run at 1 cycle/row vs 4
for fp32, the DVE one-hot builds run 2x, and all gather/collective bytes
halve. psum accumulation stays fp32; the MLP head stays fp32.
"""
import numpy as np
import concourse.bass as bass
import concourse.bacc as bacc
import concourse.mybir as mybir
import concourse.tile as tile
from concourse.library_config import mlp as mlp_lib
from concourse.masks import make_identity
from concourse.bass_utils import run_bass_kernel_spmd

F32 = mybir.dt.float32
F16 = mybir.dt.float16
I16 = mybir.dt.int16
AF = mybir.ActivationFunctionType
ALU = mybir.AluOpType

CFG = dict(N=50000, E=800000, G=256, DIN=64, DH=128, R=4, NLOC=12800,
           SUPW=2, GMAX=1536, HALF=25600, NQ=4)


def _fill_cfg(cfg):
    c = dict(cfg)
    c["NTOT"] = c["R"] * c["NLOC"]
    c["NW"] = c["NLOC"] // 128
    return c


def _edge_norms(cfg, src, dst):
    N = cfg["N"]
    deg_out = np.bincount(src, minlength=N).astype(np.float64)
    deg_in = np.bincount(dst, minlength=N).astype(np.float64)
    nsrc = np.clip(deg_out, 1.0, None) ** -0.5
    ndst = np.clip(deg_in, 1.0, None) ** -0.5
    return (nsrc[src] * ndst[dst]).astype(np.float32)


def _rank_edges(cfg, src, dst, w_all, rank):
    NLOC, HALF = cfg["NLOC"], cfg["HALF"]
    lo, hi = rank * NLOC, (rank + 1) * NLOC
    sel = (dst >= lo) & (dst < hi)
    s, d, w = src[sel], dst[sel], w_all[sel]
    order = np.argsort(d, kind="stable")
    s, d, w = s[order], d[order], w[order]
    win = (d - lo) // 128
    half = s // HALF
    out = {}
    for wi in np.unique(win):
        m = win == wi
        for h in (0, 1):
            mh = m & (half == h)
            if mh.any():
                out[(int(wi), h)] = (s[mh],
                                     (d[mh] - lo - wi * 128).astype(np.float32),
                                     w[mh])
    return out


def _build_schedule(cfg, per_core_edges):
    """Cells are (super-window, half): the SUPW consecutive windows' edges are
    packed contiguously (dst_rel relative to the super-window base, so values
    in [0, SUPW*128)) and padded once per cell. Each super-window accumulates
    into one PSUM bank [128, SUPW*128]."""
    NW, SUPW, GMAX = cfg["NW"], cfg["SUPW"], cfg["GMAX"]
    nsw = (NW + SUPW - 1) // SUPW
    nch_sh = {}
    for si in range(nsw):
        wins = range(si * SUPW, min((si + 1) * SUPW, NW))
        for h in (0, 1):
            mx = 0
            for pc in per_core_edges:
                tot = sum(len(pc[(w, h)][0]) for w in wins if (w, h) in pc)
                mx = max(mx, (tot + 127) // 128)
            if mx:
                nch_sh[(si, h)] = mx

    chunk_sw = []
    sw_instrs = []
    pos = 0
    for si in range(nsw):
        il = []
        for h in (0, 1):
            k = nch_sh.get((si, h), 0)
            chunk_sw.extend([si] * k)
            run_start = pos
            pos += k * 128
            st = run_start
            while st < pos:
                n = min(GMAX, pos - st)
                il.append((h, st, n))
                st += n
        sw_instrs.append(il)
    rows = pos
    chunk_sw = np.asarray(chunk_sw, np.int64)
    last_chunk = np.full(nsw, -1, np.int64)
    for c, si in enumerate(chunk_sw):
        last_chunk[si] = c

    per_core = []
    for pc in per_core_edges:
        gsrc = np.zeros(rows, np.int64)
        drel = np.full(rows, -1.0, np.float32)
        wv = np.zeros(rows, np.float32)
        p = 0
        for si in range(nsw):
            wins = range(si * SUPW, min((si + 1) * SUPW, NW))
            for h in (0, 1):
                k = nch_sh.get((si, h), 0)
                if not k:
                    continue
                cell_end = p + k * 128
                for wi in wins:
                    if (wi, h) in pc:
                        s, dr, w = pc[(wi, h)]
                        n = len(s)
                        gsrc[p:p + n] = s
                        drel[p:p + n] = dr + (wi - si * SUPW) * 128
                        wv[p:p + n] = w
                        p += n
                assert p <= cell_end
                p = cell_end
        assert p == rows
        per_core.append(dict(gsrc=gsrc, drel=drel, w=wv))
    sched = dict(rows=rows, chunk_sw=chunk_sw, sw_instrs=sw_instrs,
                 last_chunk=last_chunk)
    return sched, per_core


def _wrap_idx16(gsrc, half_size):
    rows = len(gsrc)
    rel = gsrc % half_size
    # tables are stored permuted ([p, c, d] SBUF-dump order): node n sits at
    # table row (n % 128) * (half_size // 128) + n // 128
    rel = ((rel % 128) * (half_size // 128) + rel // 128).astype(np.int16)
    blk = rel.reshape(rows // 16, 16).T
    return np.tile(blk, (8, 1)).copy()


def _mat128(vec):
    rows = len(vec)
    return np.ascontiguousarray(vec.reshape(rows // 128, 128).T)


def _build_nc(cfg, sched, b2_const, n_cores, nrep=1):
    N, E, G, DIN, DH, R, NLOC, NW, SUPW, GMAX, HALF, NTOT, NQ = (
        cfg[k] for k in ("N", "E", "G", "DIN", "DH", "R", "NLOC", "NW",
                         "SUPW", "GMAX", "HALF", "NTOT", "NQ"))
    ROWS = sched["rows"]
    NCH = ROWS // 128
    chunk_sw = sched["chunk_sw"]
    last_chunk = sched["last_chunk"]
    sw_instrs = sched["sw_instrs"]

    nc = bacc.Bacc("TRN2", target_bir_lowering=False, debug=False,
                   enable_asserts=True, num_devices=n_cores,
                   num_swdge_queues=NQ)

    def dram(name, shape, dt=F32, kind="ExternalInput"):
        return nc.dram_tensor(name, shape, dt, kind=kind).ap()

    xT = dram("xT", [DIN, NTOT], F16)
    gidx = dram("gidx", [128, ROWS // 16], I16)
    wmat = dram("wmat", [128, NCH], F16)
    drmat = dram("drmat", [128, NCH], F16)
    iota = dram("iota", [128, SUPW * 128], F16)
    iotaG = dram("iotaG", [128, G], F16)
    gidrow = dram("gidrow", [128, NW], F16)
    ones_row = dram("ones_row", [1, SUPW * 128], F16)
    W0 = dram("W0", [DIN, DH], F16)
    W1 = dram("W1", [DH, DH], F16)
    W2 = dram("W2", [DH, DH], F16)
    biases = dram("biases", [3, DH], F16)
    mW0su = dram("mW0su", [DH, 128])
    mW0sv = dram("mW0sv", [DH, 128])
    mW0gf = dram("mW0gf", [4, 128])
    mW1 = dram("mW1", [128, 64])
    mW2 = dram("mW2", [64, 1])
    b0c = dram("b0c", [128, 1])
    b1c = dram("b1c", [64, 1])
    gfT = dram("gfT", [4, G])
    icnt_su = dram("icnt_su", [128, G])
    icnt_sv = dram("icnt_sv", [128, G])
    y = dram("y", [G, 1], kind="ExternalOutput")

    with tile.TileContext(nc) as tc:
        with tc.tile_pool(name="const", bufs=1) as cpool, \
             tc.tile_pool(name="hT", bufs=1) as hpool, \
             tc.tile_pool(name="gath", bufs=4) as gpool, \
             tc.tile_pool(name="oneh", bufs=2) as opool, \
             tc.tile_pool(name="dense", bufs=4) as dpool, \
             tc.tile_pool(name="psc", bufs=4, space="PSUM") as psc, \
             tc.tile_pool(name="psd", bufs=4, space="PSUM") as psd, \
             tc.tile_pool(name="hd", bufs=1) as hdpool, \
             tc.tile_pool(name="dram", bufs=1, space="DRAM") as drp:

            nc.gpsimd.load_library(mlp_lib)

            t_gidx = cpool.tile([128, ROWS // 16], I16)
            nc.sync.dma_start(out=t_gidx[:], in_=gidx[:])
            t_w = cpool.tile([128, NCH], F16)
            nc.sync.dma_start(out=t_w[:], in_=wmat[:])
            t_dr = cpool.tile([128, NCH], F16)
            nc.sync.dma_start(out=t_dr[:], in_=drmat[:])
            t_iota = cpool.tile([128, SUPW * 128], F16)
            nc.sync.dma_start(out=t_iota[:], in_=iota[:])
            t_iotaG = cpool.tile([128, G], F16)
            nc.sync.dma_start(out=t_iotaG[:], in_=iotaG[:])
            t_gidrow = cpool.tile([128, NW], F16)
            nc.sync.dma_start(out=t_gidrow[:], in_=gidrow[:])
            t_ones = cpool.tile([1, SUPW * 128], F16)
            nc.sync.dma_start(out=t_ones[:], in_=ones_row[:])
            t_ident = cpool.tile([128, 128], F16)
            make_identity(nc, t_ident[:])
            t_W = []
            for nm, ap_, k in (("w0", W0, DIN), ("w1", W1, DH), ("w2", W2, DH)):
                tw = cpool.tile([k, DH], F16, name=f"t_{nm}")
                nc.sync.dma_start(out=tw[:], in_=ap_[:])
                t_W.append(tw)
            t_bias = []
            qn_state = [0]

            def one_pass(rep):
                sfx = f"_{rep}"
                for l in range(3):
                    tb_l = cpool.tile([1, DH], F16, name=f"t_bias{l}")
                    nc.sync.dma_start(out=tb_l[:], in_=biases[l:l + 1, :])
                    t_bias.append(tb_l)

                t_hT = hpool.tile([128, NLOC], F16)

                # b tables are stored permuted, in SBUF-dump order [p, c, d]:
                # node n (within a half) lives at table row (n%128)*CHALF + n//128,
                # so the dense write is 1KB-contiguous per partition and the
                # (host-remapped) gather indices absorb the permutation.
                CHALF = HALF // 128
                btbl = [[drp.tile([128, CHALF, DH], F16, name=f"btbl{i}h{h}")
                         for h in range(2)]
                    for i in range(2)]
                # NSEG=5 measured optimal: finer segments (e.g. one per
                # super-window) lose more to per-collective floors than the
                # layer-boundary overlap gains.
                NSEG = 5
                SEGW = NW // NSEG
                SEGN = SEGW * 128
                cinq = [drp.tile([128, SEGN], F16, name=f"cin{q}" + sfx)
                        for q in range(NSEG)]
                hTall = [[drp.tile([R, 128, SEGN], F16, name=f"hTall{i}q{q}" + sfx)
                          for q in range(NSEG)] for i in range(2)]
                pool_cin = drp.tile([128, G], F32, name="pool_cin")
                pool_out = drp.tile([128, G], F32, name="pool_out")
                pair_cin = drp.tile([128, G], F32, name="pair_cin")
                pair_out = drp.tile([2, 128, G], F32, name="pair_out")

                group_a = [list(range(R)), list(range(R, 2 * R))]
                group_pairs = [[r, r + R] for r in range(R)]

                def dense(l):
                    W = t_W[l]
                    K = DIN if l == 0 else DH
                    tbl = btbl[l % 2]
                    for rb in range(R):
                        for c5 in range(NLOC // 512):
                                th = dpool.tile([K, 512], F16, name="th", tag="th")
                                if l == 0:
                                    nc.sync.dma_start(
                                        out=th[:],
                                        in_=xT[:, rb * NLOC + c5 * 512:
                                                   rb * NLOC + (c5 + 1) * 512])
                                else:
                                    q = (c5 * 512) // SEGN
                                    off = (c5 * 512) % SEGN
                                    nc.sync.dma_start(
                                        out=th[:],
                                        in_=hTall[(l - 1) % 2][q][rb, :, off:off + 512])
                                tb = dpool.tile([128, 512], F16, name="tb", tag="tb")
                                pd = psd.tile([128, 512], F32, name="pd", tag="pd")
                                for j in range(4):
                                    nc.tensor.matmul(out=pd[:, j * 128:(j + 1) * 128],
                                                         lhsT=th[:, j * 128:(j + 1) * 128],
                                                         rhs=W[:], start=True, stop=True)
                                nc.scalar.activation(out=tb[:], in_=pd[:],
                                                     func=AF.Copy)
                                gbase = rb * NLOC + c5 * 512
                                tb_h = tbl[gbase // HALF]
                                base_c = (gbase % HALF) // 128
                                nc.sync.dma_start(
                                    out=tb_h.tensor.ap()[:, base_c:base_c + 4, :],
                                    in_=tb[:].rearrange("p (c d) -> p c d", c=4))

                def scatter(l, do_ag):
                    tbl = btbl[l % 2]
                    WSW = SUPW * 128
                    for si, sw0 in enumerate(range(0, NW, SUPW)):
                        ps = psc.tile([128, WSW], F32, name="ps", tag="pw")
                        nc.tensor.matmul(out=ps[:], lhsT=t_bias[l][:],
                                         rhs=t_ones[:], start=True,
                                         stop=bool(last_chunk[si] < 0))
                        for (half, st, n) in sw_instrs[si]:
                            k = n // 128
                            tg = gpool.tile([128, GMAX // 128, 128], F16,
                                            name="tg", tag="tg")
                            nc.gpsimd.dma_gather(
                                out_ap=tg[:, :k, :],
                                in_ap=tbl[half].tensor.ap()
                                    .rearrange("p c d -> (p c) d"),
                                idxs_ap=t_gidx[:, st // 16:(st + n) // 16],
                                num_idxs=n, num_idxs_reg=n, elem_size=DH,
                                single_packet=False,
                                queue_num=qn_state[0])
                            qn_state[0] = (qn_state[0] + 1) % NQ
                            nc.vector.tensor_tensor(
                                out=tg[:, :k, :], in0=tg[:, :k, :],
                                in1=t_w[:, st // 128:st // 128 + k, None]
                                    .to_broadcast([128, k, 128]),
                                op=ALU.mult)
                            toh = opool.tile([128, GMAX // 128, WSW], F16,
                                             name="toh", tag="toh")
                            nc.vector.tensor_tensor(
                                out=toh[:, :k, :],
                                in0=t_dr[:, st // 128:st // 128 + k, None]
                                    .to_broadcast([128, k, WSW]),
                                in1=t_iota[:, None, :].to_broadcast([128, k, WSW]),
                                op=ALU.is_equal)
                            for j in range(k):
                                ch = st // 128 + j
                                nc.tensor.matmul(out=ps[:], lhsT=tg[:, j, :],
                                                 rhs=toh[:, j, :], start=False,
                                                 stop=bool(ch == last_chunk[si]))
                        nc.scalar.activation(
                            out=t_hT[:, sw0 * 128:sw0 * 128 + WSW], in_=ps[:],
                            func=AF.Relu)
                        if do_ag and (sw0 * 128 + WSW) % SEGN == 0:
                            q = (sw0 * 128 + WSW) // SEGN - 1
                            nc.sync.dma_start(
                                out=cinq[q][:],
                                in_=t_hT[:, q * SEGN:(q + 1) * SEGN])
                            nc.gpsimd.collective_compute(
                                "AllGather", ALU.bypass,
                                replica_groups=group_a,
                                ins=[cinq[q][:]],
                                outs=[hTall[l % 2][q][:]])
                        if not do_ag:
                            # layer 3: pool this super-window's windows now
                            for wi in range(sw0, min(sw0 + SUPW, NW)):
                                ptr = psd.tile([128, 128], F32,
                                               name="ptr" + sfx, tag="pd")
                                nc.tensor.matmul(
                                    out=ptr[:],
                                    lhsT=t_hT[:, wi * 128:(wi + 1) * 128],
                                    rhs=t_ident[:], start=True, stop=True)
                                t_hrow = dpool.tile([128, 128], F16,
                                                    name="t_hrow" + sfx, tag="th")
                                nc.scalar.activation(out=t_hrow[:], in_=ptr[:],
                                                     func=AF.Copy)
                                t_memb = dpool.tile([128, G], F16,
                                                    name="t_memb" + sfx, tag="tb")
                                nc.vector.tensor_tensor(
                                    out=t_memb[:],
                                    in0=t_gidrow[:, wi:wi + 1]
                                        .to_broadcast([128, G]),
                                    in1=t_iotaG[:], op=ALU.is_equal)
                                nc.tensor.matmul(
                                    out=ppool[:], lhsT=t_hrow[:], rhs=t_memb[:],
                                    start=wi == 0, stop=wi == NW - 1)

                ppool = psd.tile([128, G], F32, name="ppool" + sfx, tag="pd")
                for l in range(3):
                    dense(l)
                    scatter(l, do_ag=l < 2)

                t_pool = hdpool.tile([128, G], F32, name="t_pool" + sfx, tag="t_pool")
                nc.scalar.activation(out=t_pool[:], in_=ppool[:], func=AF.Copy)
                nc.sync.dma_start(out=pool_cin[:], in_=t_pool[:])
                nc.gpsimd.collective_compute(
                    "AllReduce", ALU.add, replica_groups=group_a,
                    ins=[pool_cin[:]], outs=[pool_out[:]])
                t_pool2 = hdpool.tile([128, G], F32, name="t_pool2" + sfx, tag="t_pool2")
                nc.sync.dma_start(out=t_pool2[:], in_=pool_out[:])
                nc.sync.dma_start(out=pair_cin[:], in_=t_pool2[:])
                nc.gpsimd.collective_compute(
                    "AllGather", ALU.bypass, replica_groups=group_pairs,
                    ins=[pair_cin[:]], outs=[pair_out[:]])

                t_su = hdpool.tile([128, G], F32, name="t_su" + sfx, tag="t_su")
                t_sv = hdpool.tile([128, G], F32, name="t_sv" + sfx, tag="t_sv")
                t_icsu = hdpool.tile([128, G], F32, name="t_icsu" + sfx, tag="t_icsu")
                nc.sync.dma_start(out=t_icsu[:], in_=icnt_su[:])
                t_icsv = hdpool.tile([128, G], F32, name="t_icsv" + sfx, tag="t_icsv")
                nc.sync.dma_start(out=t_icsv[:], in_=icnt_sv[:])
                t_su_raw = hdpool.tile([128, G], F32, name="t_su_raw" + sfx, tag="t_su_raw")
                nc.sync.dma_start(out=t_su_raw[:], in_=pair_out[0])
                t_sv_raw = hdpool.tile([128, G], F32, name="t_sv_raw" + sfx, tag="t_sv_raw")
                nc.sync.dma_start(out=t_sv_raw[:], in_=pair_out[1])
                nc.vector.tensor_tensor(out=t_su[:], in0=t_su_raw[:], in1=t_icsu[:],
                                                op=ALU.mult)
                nc.vector.tensor_tensor(out=t_sv[:], in0=t_sv_raw[:], in1=t_icsv[:],
                                                op=ALU.mult)
                t_gf = hdpool.tile([4, G], F32, name="t_gf" + sfx, tag="t_gf")
                nc.sync.dma_start(out=t_gf[:], in_=gfT[:])
                t_mW0su = hdpool.tile([DH, 128], F32, name="t_mW0su" + sfx, tag="t_mW0su")
                nc.sync.dma_start(out=t_mW0su[:], in_=mW0su[:])
                t_mW0sv = hdpool.tile([DH, 128], F32, name="t_mW0sv" + sfx, tag="t_mW0sv")
                nc.sync.dma_start(out=t_mW0sv[:], in_=mW0sv[:])
                t_mW0gf = hdpool.tile([4, 128], F32, name="t_mW0gf" + sfx, tag="t_mW0gf")
                nc.sync.dma_start(out=t_mW0gf[:], in_=mW0gf[:])
                t_mW1 = hdpool.tile([128, 64], F32, name="t_mW1" + sfx, tag="t_mW1")
                nc.sync.dma_start(out=t_mW1[:], in_=mW1[:])
                t_mW2 = hdpool.tile([64, 1], F32, name="t_mW2" + sfx, tag="t_mW2")
                nc.sync.dma_start(out=t_mW2[:], in_=mW2[:])
                t_b0c = hdpool.tile([128, 1], F32, name="t_b0c" + sfx, tag="t_b0c")
                nc.sync.dma_start(out=t_b0c[:], in_=b0c[:])
                t_b1c = hdpool.tile([64, 1], F32, name="t_b1c" + sfx, tag="t_b1c")
                nc.sync.dma_start(out=t_b1c[:], in_=b1c[:])

                ph1 = psd.tile([128, G], F32, name="ph1" + sfx, tag="pd")
                nc.tensor.matmul(out=ph1[:], lhsT=t_mW0su[:], rhs=t_su[:],
                                     start=True, stop=False)
                nc.tensor.matmul(out=ph1[:], lhsT=t_mW0sv[:], rhs=t_sv[:],
                                     start=False, stop=False)
                nc.tensor.matmul(out=ph1[:], lhsT=t_mW0gf[:], rhs=t_gf[:],
                                     start=False, stop=True)
                t_h1 = hdpool.tile([128, G], F32, name="t_h1" + sfx, tag="t_h1")
                nc.scalar.activation(out=t_h1[:], in_=ph1[:], func=AF.Relu,
                                         bias=t_b0c[:, :1])
                ph2 = psd.tile([64, G], F32, name="ph2" + sfx, tag="pd")
                nc.tensor.matmul(out=ph2[:], lhsT=t_mW1[:], rhs=t_h1[:],
                                     start=True, stop=True)
                t_h2 = hdpool.tile([64, G], F32, name="t_h2" + sfx, tag="t_h2")
                nc.scalar.activation(out=t_h2[:], in_=ph2[:], func=AF.Relu,
                                         bias=t_b1c[:, :1])
                po = psd.tile([1, G], F32, name="po" + sfx, tag="pd")
                nc.tensor.matmul(out=po[:], lhsT=t_mW2[:], rhs=t_h2[:],
                                     start=True, stop=True)
                t_o = hdpool.tile([1, G], F32, name="t_o" + sfx, tag="t_o")
                nc.scalar.activation(out=t_o[:], in_=po[:], func=AF.Copy,
                                         bias=float(b2_const))
                nc.sync.dma_start(out=y[:], in_=t_o[:, :, None])

            for rep in range(nrep):
                one_pass(rep)

    nc.compile()
    return nc


def _host_prep(cfg, inputs):
    cfg = _fill_cfg(cfg)
    N, G, DIN, DH, R, NLOC, NW, NTOT, HALF = (
        cfg[k] for k in ("N", "G", "DIN", "DH", "R", "NLOC", "NW", "NTOT",
                         "HALF"))
    enc = []
    for pre in ("solute", "solvent"):
        src = np.asarray(inputs[f"{pre}_src"]).astype(np.int64)
        dst = np.asarray(inputs[f"{pre}_dst"]).astype(np.int64)
        gid = np.asarray(inputs[f"{pre}_gid"]).astype(np.int64)
        x = np.asarray(inputs[f"{pre}_x"], np.float32)
        w_all = _edge_norms(cfg, src, dst)
        enc.append(dict(src=src, dst=dst, gid=gid, x=x, w=w_all))

    per_core_edges = []
    for e in enc:
        for r in range(R):
            per_core_edges.append(_rank_edges(cfg, e["src"], e["dst"], e["w"], r))
    sched, pc_arrays = _build_schedule(cfg, per_core_edges)

    iota = np.broadcast_to(np.arange(cfg["SUPW"] * 128, dtype=np.float16),
                           (128, cfg["SUPW"] * 128)).copy()
    iotaG = np.broadcast_to(np.arange(G, dtype=np.float16), (128, G)).copy()
    ones_row = np.ones((1, cfg["SUPW"] * 128), np.float16)
    gfT = np.ascontiguousarray(np.asarray(inputs["global_feats"], np.float32).T)
    mW0 = np.asarray(inputs["mlp_W0"], np.float32)
    icnts = []
    for e in enc:
        cnt = np.maximum(np.bincount(e["gid"], minlength=G), 1.0).astype(np.float32)
        icnts.append(np.broadcast_to(1.0 / cnt, (128, G)).copy())
    b2_const = float(np.asarray(inputs["mlp_b2"]).reshape(-1)[0])

    xTs, gidrows = [], []
    for e in enc:
        xp = np.zeros((NTOT, DIN), np.float32)
        xp[:N] = e["x"]
        xTs.append(np.ascontiguousarray(xp.T).astype(np.float16))
        gr = np.full(NTOT, -1.0, np.float32)
        gr[:N] = e["gid"].astype(np.float32)
        gidrows.append(gr)

    in_maps = []
    for gi in range(2):
        pre = "su" if gi == 0 else "sv"
        for r in range(R):
            c = gi * R + r
            arr = pc_arrays[c]
            gr_loc = gidrows[gi][r * NLOC:(r + 1) * NLOC]
            im = dict(
                xT=xTs[gi],
                gidx=_wrap_idx16(arr["gsrc"], HALF),
                wmat=_mat128(arr["w"]).astype(np.float16),
                drmat=_mat128(arr["drel"]).astype(np.float16),
                iota=iota, iotaG=iotaG,
                gidrow=_mat128(gr_loc).astype(np.float16),
                ones_row=ones_row,
                W0=np.asarray(inputs[f"{pre}_W0"], np.float32).astype(np.float16),
                W1=np.asarray(inputs[f"{pre}_W1"], np.float32).astype(np.float16),
                W2=np.asarray(inputs[f"{pre}_W2"], np.float32).astype(np.float16),
                biases=np.asarray(inputs[f"{pre}_b"], np.float32).astype(np.float16),
                mW0su=np.ascontiguousarray(mW0[0:DH, :]),
                mW0sv=np.ascontiguousarray(mW0[DH:2 * DH, :]),
                mW0gf=np.ascontiguousarray(mW0[2 * DH:2 * DH + 4, :]),
                mW1=np.asarray(inputs["mlp_W1"], np.float32),
                mW2=np.asarray(inputs["mlp_W2"], np.float32),
                b0c=np.asarray(inputs["mlp_b0"], np.float32).reshape(128, 1),
                b1c=np.asarray(inputs["mlp_b1"], np.float32).reshape(64, 1),
                gfT=gfT, icnt_su=icnts[0], icnt_sv=icnts[1],
            )
            in_maps.append(im)
    return cfg, sched, b2_const, in_maps


_CACHE = {}


def kernel(**inputs) -> np.ndarray:
    cfg, sched, b2c, in_maps = _host_prep(CFG, inputs)
    key = (sched["rows"], b2c, sched["chunk_sw"].tobytes(),
           tuple(i for sw in sched["sw_instrs"] for i in sw))
    nc = _CACHE.get(key)
    if nc is None:
        nc = _build_nc(cfg, sched, b2c, 8)
        _CACHE[key] = nc
    res = run_bass_kernel_spmd(nc, in_maps, core_ids=list(range(8)))
    return np.asarray(res.results[0]["y"], np.float32)


# revision 9
# speedup vs baseline: 1.3569x; 1.3569x over previous
"""DualGNNModel Trainium2 kernel (8 NeuronCores, Bass/Tile).

Self-contained: accepts FULL inputs (as reference.setup_inputs()), returns the
FULL [256, 1] float32 output.

Sharding: cores 0-3 run the solute GCN encoder, cores 4-7 the solvent encoder
(graph-level model parallelism over the two independent encoders). Within each
4-core group, edges are partitioned by destination node into 4 contiguous
12800-node ranges; GCN/MLP weights are replicated. Per layer each core:
  dense   b = h @ W                  (replicated over the group, fp16)
  gather  g = b[src] rows            (dma_gather, two int16 half-table passes,
                                      fp16 rows, round-robin over 4 SWDGE
                                      queues for HW DMA parallelism)
  scale   g *= nsrc[src]*ndst[dst]   (symmetric GCN norm folded per edge)
  scatter psum[feat, dstwin] += g_chunk^T @ onehot(dst)    (fp16 PE matmuls)
  bias    psum += outer(bias, ones);  hT_own = relu(psum)
h shards are AllGather'd between layers; after layer 3 each core pools its own
shard via a membership-one-hot matmul, partial pools are AllReduce'd within the
group, the two encoders' pooled embeddings are exchanged pairwise, and the
(tiny) MLP head runs replicated on every core.

The edge datapath is fp16 end to end (gather table, edge norms, one-hot
destination masks, PE scatter matmuls): fp16 matmuls run at 1 cycle/row vs 4
for fp32, the DVE one-hot builds run 2x, and all gather/collective bytes
halve. psum accumulation stays fp32; the MLP head stays fp32.
"""
import numpy as np
import concourse.bass as bass
import concourse.bacc as bacc
import concourse.mybir as mybir
import concourse.tile as tile
from concourse.library_config import mlp as mlp_lib
from concourse.masks import make_identity
from concourse.bass_utils import run_bass_kernel_spmd

F32 = mybir.dt.float32
F16 = mybir.dt.float16
I16 = mybir.dt.int16
AF = mybir.ActivationFunctionType
ALU = mybir.AluOpType

CFG = dict(N=50000, E=800000, G=256, DIN=64, DH=128, R=4, NLOC=12800,
           SUPW=1, GMAX=1536, HALF=25600, NQ=4)


def _fill_cfg(cfg):
    c = dict(cfg)
    c["NTOT"] = c["R"] * c["NLOC"]
    c["NW"] = c["NLOC"] // 128
    return c


def _edge_norms(cfg, src, dst):
    N = cfg["N"]
    deg_out = np.bincount(src, minlength=N).astype(np.float64)
    deg_in = np.bincount(dst, minlength=N).astype(np.float64)
    nsrc = np.clip(deg_out, 1.0, None) ** -0.5
    ndst = np.clip(deg_in, 1.0, None) ** -0.5
    return (nsrc[src] * ndst[dst]).astype(np.float32)


def _rank_edges(cfg, src, dst, w_all, rank):
    NLOC, HALF = cfg["NLOC"], cfg["HALF"]
    lo, hi = rank * NLOC, (rank + 1) * NLOC
    sel = (dst >= lo) & (dst < hi)
    s, d, w = src[sel], dst[sel], w_all[sel]
    order = np.argsort(d, kind="stable")
    s, d, w = s[order], d[order], w[order]
    win = (d - lo) // 128
    half = s // HALF
    out = {}
    for wi in np.unique(win):
        m = win == wi
        for h in (0, 1):
            mh = m & (half == h)
            if mh.any():
                out[(int(wi), h)] = (s[mh],
                                     (d[mh] - lo - wi * 128).astype(np.float32),
                                     w[mh])
    return out


def _build_schedule(cfg, per_core_edges):
    """Cells are (super-window, half): the SUPW consecutive windows' edges are
    packed contiguously (dst_rel relative to the super-window base, so values
    in [0, SUPW*128)) and padded once per cell. Each super-window accumulates
    into one PSUM bank [128, SUPW*128]."""
    NW, SUPW, GMAX = cfg["NW"], cfg["SUPW"], cfg["GMAX"]
    nsw = (NW + SUPW - 1) // SUPW
    nch_sh = {}
    for si in range(nsw):
        wins = range(si * SUPW, min((si + 1) * SUPW, NW))
        for h in (0, 1):
            mx = 0
            for pc in per_core_edges:
                tot = sum(len(pc[(w, h)][0]) for w in wins if (w, h) in pc)
                mx = max(mx, (tot + 127) // 128)
            if mx:
                nch_sh[(si, h)] = mx

    chunk_sw = []
    sw_instrs = []
    pos = 0
    for si in range(nsw):
        il = []
        for h in (0, 1):
            k = nch_sh.get((si, h), 0)
            chunk_sw.extend([si] * k)
            run_start = pos
            pos += k * 128
            st = run_start
            while st < pos:
                n = min(GMAX, pos - st)
                il.append((h, st, n))
                st += n
        sw_instrs.append(il)
    rows = pos
    chunk_sw = np.asarray(chunk_sw, np.int64)
    last_chunk = np.full(nsw, -1, np.int64)
    for c, si in enumerate(chunk_sw):
        last_chunk[si] = c

    per_core = []
    for pc in per_core_edges:
        gsrc = np.zeros(rows, np.int64)
        drel = np.full(rows, -1.0, np.float32)
        wv = np.zeros(rows, np.float32)
        p = 0
        for si in range(nsw):
            wins = range(si * SUPW, min((si + 1) * SUPW, NW))
            for h in (0, 1):
                k = nch_sh.get((si, h), 0)
                if not k:
                    continue
                cell_end = p + k * 128
                for wi in wins:
                    if (wi, h) in pc:
                        s, dr, w = pc[(wi, h)]
                        n = len(s)
                        gsrc[p:p + n] = s
                        drel[p:p + n] = dr + (wi - si * SUPW) * 128
                        wv[p:p + n] = w
                        p += n
                assert p <= cell_end
                p = cell_end
        assert p == rows
        per_core.append(dict(gsrc=gsrc, drel=drel, w=wv))
    sched = dict(rows=rows, chunk_sw=chunk_sw, sw_instrs=sw_instrs,
                 last_chunk=last_chunk)
    return sched, per_core


def _wrap_idx16(gsrc, half_size):
    rows = len(gsrc)
    rel = gsrc % half_size
    # tables are stored permuted ([p, c, d] SBUF-dump order): node n sits at
    # table row (n % 128) * (half_size // 128) + n // 128
    rel = ((rel % 128) * (half_size // 128) + rel // 128).astype(np.int16)
    blk = rel.reshape(rows // 16, 16).T
    return np.tile(blk, (8, 1)).copy()


def _mat128(vec):
    rows = len(vec)
    return np.ascontiguousarray(vec.reshape(rows // 128, 128).T)


def _build_nc(cfg, sched, b2_const, n_cores, nrep=1):
    N, E, G, DIN, DH, R, NLOC, NW, SUPW, GMAX, HALF, NTOT, NQ = (
        cfg[k] for k in ("N", "E", "G", "DIN", "DH", "R", "NLOC", "NW",
                         "SUPW", "GMAX", "HALF", "NTOT", "NQ"))
    ROWS = sched["rows"]
    NCH = ROWS // 128
    chunk_sw = sched["chunk_sw"]
    last_chunk = sched["last_chunk"]
    sw_instrs = sched["sw_instrs"]

    nc = bacc.Bacc("TRN2", target_bir_lowering=False, debug=False,
                   enable_asserts=True, num_devices=n_cores,
                   num_swdge_queues=NQ)

    def dram(name, shape, dt=F32, kind="ExternalInput"):
        return nc.dram_tensor(name, shape, dt, kind=kind).ap()

    xT = dram("xT", [DIN, NTOT], F16)
    gidx = dram("gidx", [128, ROWS // 16], I16)
    wmat = dram("wmat", [128, NCH], F16)
    drmat = dram("drmat", [128, NCH], F16)
    iota = dram("iota", [128, SUPW * 128], F16)
    iotaG = dram("iotaG", [128, G], F16)
    gidrow = dram("gidrow", [128, NW], F16)
    ones_row = dram("ones_row", [1, SUPW * 128], F16)
    W0 = dram("W0", [DIN, DH], F16)
    W1 = dram("W1", [DH, DH], F16)
    W2 = dram("W2", [DH, DH], F16)
    biases = dram("biases", [3, DH], F16)
    mW0su = dram("mW0su", [DH, 128])
    mW0sv = dram("mW0sv", [DH, 128])
    mW0gf = dram("mW0gf", [4, 128])
    mW1 = dram("mW1", [128, 64])
    mW2 = dram("mW2", [64, 1])
    b0c = dram("b0c", [128, 1])
    b1c = dram("b1c", [64, 1])
    gfT = dram("gfT", [4, G])
    icnt_su = dram("icnt_su", [128, G])
    icnt_sv = dram("icnt_sv", [128, G])
    y = dram("y", [G, 1], kind="ExternalOutput")

    with tile.TileContext(nc) as tc:
        with tc.tile_pool(name="const", bufs=1) as cpool, \
             tc.tile_pool(name="hT", bufs=1) as hpool, \
             tc.tile_pool(name="gath", bufs=4) as gpool, \
             tc.tile_pool(name="oneh", bufs=2) as opool, \
             tc.tile_pool(name="dense", bufs=4) as dpool, \
             tc.tile_pool(name="psc", bufs=4, space="PSUM") as psc, \
             tc.tile_pool(name="psd", bufs=4, space="PSUM") as psd, \
             tc.tile_pool(name="hd", bufs=1) as hdpool, \
             tc.tile_pool(name="dram", bufs=1, space="DRAM") as drp:

            nc.gpsimd.load_library(mlp_lib)

            t_gidx = cpool.tile([128, ROWS // 16], I16)
            nc.sync.dma_start(out=t_gidx[:], in_=gidx[:])
            t_w = cpool.tile([128, NCH], F16)
            nc.sync.dma_start(out=t_w[:], in_=wmat[:])
            t_dr = cpool.tile([128, NCH], F16)
            nc.sync.dma_start(out=t_dr[:], in_=drmat[:])
            t_iota = cpool.tile([128, SUPW * 128], F16)
            nc.sync.dma_start(out=t_iota[:], in_=iota[:])
            t_iotaG = cpool.tile([128, G], F16)
            nc.sync.dma_start(out=t_iotaG[:], in_=iotaG[:])
            t_gidrow = cpool.tile([128, NW], F16)
            nc.sync.dma_start(out=t_gidrow[:], in_=gidrow[:])
            t_ones = cpool.tile([1, SUPW * 128], F16)
            nc.sync.dma_start(out=t_ones[:], in_=ones_row[:])
            t_ident = cpool.tile([128, 128], F16)
            make_identity(nc, t_ident[:])
            t_W = []
            for nm, ap_, k in (("w0", W0, DIN), ("w1", W1, DH), ("w2", W2, DH)):
                tw = cpool.tile([k, DH], F16, name=f"t_{nm}")
                nc.sync.dma_start(out=tw[:], in_=ap_[:])
                t_W.append(tw)
            t_bias = []
            qn_state = [0]

            def one_pass(rep):
                sfx = f"_{rep}"
                for l in range(3):
                    tb_l = cpool.tile([1, DH], F16, name=f"t_bias{l}")
                    nc.sync.dma_start(out=tb_l[:], in_=biases[l:l + 1, :])
                    t_bias.append(tb_l)

                t_hT = hpool.tile([128, NLOC], F16)

                # b tables are stored permuted, in SBUF-dump order [p, c, d]:
                # node n (within a half) lives at table row (n%128)*CHALF + n//128,
                # so the dense write is 1KB-contiguous per partition and the
                # (host-remapped) gather indices absorb the permutation.
                CHALF = HALF // 128
                btbl = [[drp.tile([128, CHALF, DH], F16, name=f"btbl{i}h{h}")
                         for h in range(2)]
                    for i in range(2)]
                # NSEG=5 measured optimal: finer segments (e.g. one per
                # super-window) lose more to per-collective floors than the
                # layer-boundary overlap gains.
                NSEG = 5
                SEGW = NW // NSEG
                SEGN = SEGW * 128
                cinq = [drp.tile([128, SEGN], F16, name=f"cin{q}" + sfx)
                        for q in range(NSEG)]
                hTall = [[drp.tile([R, 128, SEGN], F16, name=f"hTall{i}q{q}" + sfx)
                          for q in range(NSEG)] for i in range(2)]
                pool_cin = drp.tile([128, G], F32, name="pool_cin")
                pool_out = drp.tile([128, G], F32, name="pool_out")
                pair_cin = drp.tile([128, G], F32, name="pair_cin")
                pair_out = drp.tile([2, 128, G], F32, name="pair_out")

                group_a = [list(range(R)), list(range(R, 2 * R))]
                group_pairs = [[r, r + R] for r in range(R)]

                def dense(l):
                    W = t_W[l]
                    K = DIN if l == 0 else DH
                    tbl = btbl[l % 2]
                    for rb in range(R):
                        for c5 in range(NLOC // 512):
                                th = dpool.tile([K, 512], F16, name="th", tag="th")
                                if l == 0:
                                    nc.sync.dma_start(
                                        out=th[:],
                                        in_=xT[:, rb * NLOC + c5 * 512:
                                                   rb * NLOC + (c5 + 1) * 512])
                                else:
                                    q = (c5 * 512) // SEGN
                                    off = (c5 * 512) % SEGN
                                    nc.sync.dma_start(
                                        out=th[:],
                                        in_=hTall[(l - 1) % 2][q][rb, :, off:off + 512])
                                tb = dpool.tile([128, 512], F16, name="tb", tag="tb")
                                pd = psd.tile([128, 512], F32, name="pd", tag="pd")
                                for j in range(4):
                                    nc.tensor.matmul(out=pd[:, j * 128:(j + 1) * 128],
                                                         lhsT=th[:, j * 128:(j + 1) * 128],
                                                         rhs=W[:], start=True, stop=True)
                                nc.scalar.activation(out=tb[:], in_=pd[:],
                                                     func=AF.Copy)
                                gbase = rb * NLOC + c5 * 512
                                tb_h = tbl[gbase // HALF]
                                base_c = (gbase % HALF) // 128
                                nc.sync.dma_start(
                                    out=tb_h.tensor.ap()[:, base_c:base_c + 4, :],
                                    in_=tb[:].rearrange("p (c d) -> p c d", c=4))

                def scatter(l, do_ag):
                    tbl = btbl[l % 2]
                    WSW = SUPW * 128
                    for si, sw0 in enumerate(range(0, NW, SUPW)):
                        ps = psc.tile([128, WSW], F32, name="ps", tag="pw")
                        nc.tensor.matmul(out=ps[:], lhsT=t_bias[l][:],
                                         rhs=t_ones[:], start=True,
                                         stop=bool(last_chunk[si] < 0))
                        for (half, st, n) in sw_instrs[si]:
                            k = n // 128
                            tg = gpool.tile([128, GMAX // 128, 128], F16,
                                            name="tg", tag="tg")
                            nc.gpsimd.dma_gather(
                                out_ap=tg[:, :k, :],
                                in_ap=tbl[half].tensor.ap()
                                    .rearrange("p c d -> (p c) d"),
                                idxs_ap=t_gidx[:, st // 16:(st + n) // 16],
                                num_idxs=n, num_idxs_reg=n, elem_size=DH,
                                single_packet=False,
                                queue_num=qn_state[0])
                            qn_state[0] = (qn_state[0] + 1) % NQ
                            nc.vector.tensor_tensor(
                                out=tg[:, :k, :], in0=tg[:, :k, :],
                                in1=t_w[:, st // 128:st // 128 + k, None]
                                    .to_broadcast([128, k, 128]),
                                op=ALU.mult)
                            toh = opool.tile([128, GMAX // 128, WSW], F16,
                                             name="toh", tag="toh")
                            nc.vector.tensor_tensor(
                                out=toh[:, :k, :],
                                in0=t_dr[:, st // 128:st // 128 + k, None]
                                    .to_broadcast([128, k, WSW]),
                                in1=t_iota[:, None, :].to_broadcast([128, k, WSW]),
                                op=ALU.is_equal)
                            for j in range(k):
                                ch = st // 128 + j
                                nc.tensor.matmul(out=ps[:], lhsT=tg[:, j, :],
                                                 rhs=toh[:, j, :], start=False,
                                                 stop=bool(ch == last_chunk[si]))
                        nc.scalar.activation(
                            out=t_hT[:, sw0 * 128:sw0 * 128 + WSW], in_=ps[:],
                            func=AF.Relu)
                        if do_ag and (sw0 * 128 + WSW) % SEGN == 0:
                            q = (sw0 * 128 + WSW) // SEGN - 1
                            nc.sync.dma_start(
                                out=cinq[q][:],
                                in_=t_hT[:, q * SEGN:(q + 1) * SEGN])
                            nc.gpsimd.collective_compute(
                                "AllGather", ALU.bypass,
                                replica_groups=group_a,
                                ins=[cinq[q][:]],
                                outs=[hTall[l % 2][q][:]])
                        if not do_ag:
                            # layer 3: pool this super-window's windows now
                            for wi in range(sw0, min(sw0 + SUPW, NW)):
                                ptr = psd.tile([128, 128], F32,
                                               name="ptr" + sfx, tag="pd")
                                nc.tensor.matmul(
                                    out=ptr[:],
                                    lhsT=t_hT[:, wi * 128:(wi + 1) * 128],
                                    rhs=t_ident[:], start=True, stop=True)
                                t_hrow = dpool.tile([128, 128], F16,
                                                    name="t_hrow" + sfx, tag="th")
                                nc.scalar.activation(out=t_hrow[:], in_=ptr[:],
                                                     func=AF.Copy)
                                t_memb = dpool.tile([128, G], F16,
                                                    name="t_memb" + sfx, tag="tb")
                                nc.vector.tensor_tensor(
                                    out=t_memb[:],
                                    in0=t_gidrow[:, wi:wi + 1]
                                        .to_broadcast([128, G]),
                                    in1=t_iotaG[:], op=ALU.is_equal)
                                nc.tensor.matmul(
                                    out=ppool[:], lhsT=t_hrow[:], rhs=t_memb[:],
                                    start=wi == 0, stop=wi == NW - 1)

                ppool = psd.tile([128, G], F32, name="ppool" + sfx, tag="pd")
                for l in range(3):
                    dense(l)
                    scatter(l, do_ag=l < 2)

                t_pool = hdpool.tile([128, G], F32, name="t_pool" + sfx, tag="t_pool")
                nc.scalar.activation(out=t_pool[:], in_=ppool[:], func=AF.Copy)
                nc.sync.dma_start(out=pool_cin[:], in_=t_pool[:])
                nc.gpsimd.collective_compute(
                    "AllReduce", ALU.add, replica_groups=group_a,
                    ins=[pool_cin[:]], outs=[pool_out[:]])
                t_pool2 = hdpool.tile([128, G], F32, name="t_pool2" + sfx, tag="t_pool2")
                nc.sync.dma_start(out=t_pool2[:], in_=pool_out[:])
                nc.sync.dma_start(out=pair_cin[:], in_=t_pool2[:])
                nc.gpsimd.collective_compute(
                    "AllGather", ALU.bypass, replica_groups=group_pairs,
                    ins=[pair_cin[:]], outs=[pair_out[:]])

                t_su = hdpool.tile([128, G], F32, name="t_su" + sfx, tag="t_su")
                t_sv = hdpool.tile([128, G], F32, name="t_sv" + sfx, tag="t_sv")
                t_icsu = hdpool.tile([128, G], F32, name="t_icsu" + sfx, tag="t_icsu")
                nc.sync.dma_start(out=t_icsu[:], in_=icnt_su[:])
                t_icsv = hdpool.tile([128, G], F32, name="t_icsv" + sfx, tag="t_icsv")
                nc.sync.dma_start(out=t_icsv[:], in_=icnt_sv[:])
                t_su_raw = hdpool.tile([128, G], F32, name="t_su_raw" + sfx, tag="t_su_raw")
                nc.sync.dma_start(out=t_su_raw[:], in_=pair_out[0])
                t_sv_raw = hdpool.tile([128, G], F32, name="t_sv_raw" + sfx, tag="t_sv_raw")
                nc.sync.dma_start(out=t_sv_raw[:], in_=pair_out[1])
                nc.vector.tensor_tensor(out=t_su[:], in0=t_su_raw[:], in1=t_icsu[:],
                                                op=ALU.mult)
                nc.vector.tensor_tensor(out=t_sv[:], in0=t_sv_raw[:], in1=t_icsv[:],
                                                op=ALU.mult)
                t_gf = hdpool.tile([4, G], F32, name="t_gf" + sfx, tag="t_gf")
                nc.sync.dma_start(out=t_gf[:], in_=gfT[:])
                t_mW0su = hdpool.tile([DH, 128], F32, name="t_mW0su" + sfx, tag="t_mW0su")
                nc.sync.dma_start(out=t_mW0su[:], in_=mW0su[:])
                t_mW0sv = hdpool.tile([DH, 128], F32, name="t_mW0sv" + sfx, tag="t_mW0sv")
                nc.sync.dma_start(out=t_mW0sv[:], in_=mW0sv[:])
                t_mW0gf = hdpool.tile([4, 128], F32, name="t_mW0gf" + sfx, tag="t_mW0gf")
                nc.sync.dma_start(out=t_mW0gf[:], in_=mW0gf[:])
                t_mW1 = hdpool.tile([128, 64], F32, name="t_mW1" + sfx, tag="t_mW1")
                nc.sync.dma_start(out=t_mW1[:], in_=mW1[:])
                t_mW2 = hdpool.tile([64, 1], F32, name="t_mW2" + sfx, tag="t_mW2")
                nc.sync.dma_start(out=t_mW2[:], in_=mW2[:])
                t_b0c = hdpool.tile([128, 1], F32, name="t_b0c" + sfx, tag="t_b0c")
                nc.sync.dma_start(out=t_b0c[:], in_=b0c[:])
                t_b1c = hdpool.tile([64, 1], F32, name="t_b1c" + sfx, tag="t_b1c")
                nc.sync.dma_start(out=t_b1c[:], in_=b1c[:])

                ph1 = psd.tile([128, G], F32, name="ph1" + sfx, tag="pd")
                nc.tensor.matmul(out=ph1[:], lhsT=t_mW0su[:], rhs=t_su[:],
                                     start=True, stop=False)
                nc.tensor.matmul(out=ph1[:], lhsT=t_mW0sv[:], rhs=t_sv[:],
                                     start=False, stop=False)
                nc.tensor.matmul(out=ph1[:], lhsT=t_mW0gf[:], rhs=t_gf[:],
                                     start=False, stop=True)
                t_h1 = hdpool.tile([128, G], F32, name="t_h1" + sfx, tag="t_h1")
                nc.scalar.activation(out=t_h1[:], in_=ph1[:], func=AF.Relu,
                                         bias=t_b0c[:, :1])
                ph2 = psd.tile([64, G], F32, name="ph2" + sfx, tag="pd")
                nc.tensor.matmul(out=ph2[:], lhsT=t_mW1[:], rhs=t_h1[:],
                                     start=True, stop=True)
                t_h2 = hdpool.tile([64, G], F32, name="t_h2" + sfx, tag="t_h2")
                nc.scalar.activation(out=t_h2[:], in_=ph2[:], func=AF.Relu,
                                         bias=t_b1c[:, :1])
                po = psd.tile([1, G], F32, name="po" + sfx, tag="pd")
                nc.tensor.matmul(out=po[:], lhsT=t_mW2[:], rhs=t_h2[:],
                                     start=True, stop=True)
                t_o = hdpool.tile([1, G], F32, name="t_o" + sfx, tag="t_o")
                nc.scalar.activation(out=t_o[:], in_=po[:], func=AF.Copy,
                                         bias=float(b2_const))
                nc.sync.dma_start(out=y[:], in_=t_o[:, :, None])

            for rep in range(nrep):
                one_pass(rep)

    nc.compile()
    return nc


def _host_prep(cfg, inputs):
    cfg = _fill_cfg(cfg)
    N, G, DIN, DH, R, NLOC, NW, NTOT, HALF = (
        cfg[k] for k in ("N", "G", "DIN", "DH", "R", "NLOC", "NW", "NTOT",
                         "HALF"))
    enc = []
    for pre in ("solute", "solvent"):
        src = np.asarray(inputs[f"{pre}_src"]).astype(np.int64)
        dst = np.asarray(inputs[f"{pre}_dst"]).astype(np.int64)
        gid = np.asarray(inputs[f"{pre}_gid"]).astype(np.int64)
        x = np.asarray(inputs[f"{pre}_x"], np.float32)
        w_all = _edge_norms(cfg, src, dst)
        enc.append(dict(src=src, dst=dst, gid=gid, x=x, w=w_all))

    per_core_edges = []
    for e in enc:
        for r in range(R):
            per_core_edges.append(_rank_edges(cfg, e["src"], e["dst"], e["w"], r))
    sched, pc_arrays = _build_schedule(cfg, per_core_edges)

    iota = np.broadcast_to(np.arange(cfg["SUPW"] * 128, dtype=np.float16),
                           (128, cfg["SUPW"] * 128)).copy()
    iotaG = np.broadcast_to(np.arange(G, dtype=np.float16), (128, G)).copy()
    ones_row = np.ones((1, cfg["SUPW"] * 128), np.float16)
    gfT = np.ascontiguousarray(np.asarray(inputs["global_feats"], np.float32).T)
    mW0 = np.asarray(inputs["mlp_W0"], np.float32)
    icnts = []
    for e in enc:
        cnt = np.maximum(np.bincount(e["gid"], minlength=G), 1.0).astype(np.float32)
        icnts.append(np.broadcast_to(1.0 / cnt, (128, G)).copy())
    b2_const = float(np.asarray(inputs["mlp_b2"]).reshape(-1)[0])

    xTs, gidrows = [], []
    for e in enc:
        xp = np.zeros((NTOT, DIN), np.float32)
        xp[:N] = e["x"]
        xTs.append(np.ascontiguousarray(xp.T).astype(np.float16))
        gr = np.full(NTOT, -1.0, np.float32)
        gr[:N] = e["gid"].astype(np.float32)
        gidrows.append(gr)

    in_maps = []
    for gi in range(2):
        pre = "su" if gi == 0 else "sv"
        for r in range(R):
            c = gi * R + r
            arr = pc_arrays[c]
            gr_loc = gidrows[gi][r * NLOC:(r + 1) * NLOC]
            im = dict(
                xT=xTs[gi],
                gidx=_wrap_idx16(arr["gsrc"], HALF),
                wmat=_mat128(arr["w"]).astype(np.float16),
                drmat=_mat128(arr["drel"]).astype(np.float16),
                iota=iota, iotaG=iotaG,
                gidrow=_mat128(gr_loc).astype(np.float16),
                ones_row=ones_row,
                W0=np.asarray(inputs[f"{pre}_W0"], np.float32).astype(np.float16),
                W1=np.asarray(inputs[f"{pre}_W1"], np.float32).astype(np.float16),
                W2=np.asarray(inputs[f"{pre}_W2"], np.float32).astype(np.float16),
                biases=np.asarray(inputs[f"{pre}_b"], np.float32).astype(np.float16),
                mW0su=np.ascontiguousarray(mW0[0:DH, :]),
                mW0sv=np.ascontiguousarray(mW0[DH:2 * DH, :]),
                mW0gf=np.ascontiguousarray(mW0[2 * DH:2 * DH + 4, :]),
                mW1=np.asarray(inputs["mlp_W1"], np.float32),
                mW2=np.asarray(inputs["mlp_W2"], np.float32),
                b0c=np.asarray(inputs["mlp_b0"], np.float32).reshape(128, 1),
                b1c=np.asarray(inputs["mlp_b1"], np.float32).reshape(64, 1),
                gfT=gfT, icnt_su=icnts[0], icnt_sv=icnts[1],
            )
            in_maps.append(im)
    return cfg, sched, b2_const, in_maps


_CACHE = {}


def kernel(**inputs) -> np.ndarray:
    cfg, sched, b2c, in_maps = _host_prep(CFG, inputs)
    key = (sched["rows"], b2c, sched["chunk_sw"].tobytes(),
           tuple(i for sw in sched["sw_instrs"] for i in sw))
    nc = _CACHE.get(key)
    if nc is None:
        nc = _build_nc(cfg, sched, b2c, 8)
        _CACHE[key] = nc
    res = run_bass_kernel_spmd(nc, in_maps, core_ids=list(range(8)))
    return np.asarray(res.results[0]["y"], np.float32)
